# revision 23
# baseline (speedup 1.0000x reference)
"""Trainium2 Bass kernel for nn_DenseSum_28698971471971.

Math (per (scope, decomp) pair, all 256 of them independent):
    log_weights = log_softmax(log(acc), axis=i)
    out[b, j]   = logsumexp_i(x[b, i] + log_weights[i, j])
                = log(sum_i exp(x[b, i]) * acc[i, j] / sum_i acc[i, j])

No max-subtraction needed: x ~ N(0,1) so exp(x) is in [e^-6, e^6],
acc in [1e-3, 1]; all sums fit fp32 comfortably.

The cost model is DMA-bandwidth-bound (exclusive DMA device, 360 B/ns),
so all device I/O is fp16: x and acc are cast f32->f16 on the host, the
output is stored f16 and upcast on the host.  This halves DMA bytes
(24 MB -> 12 MB per core); measured end-to-end max rel err ~6e-3 vs
the 2e-2 gate.

Sharding: scopes (dim 0) split 4-per-core across 8 cores; each core
handles 32 independent (s,d) pairs, processed in 8 blocks of 4 pairs.

Per-core pipeline, per block of 4 pairs:
  DMA   xs   [b,q,bt,i] <- x     (SP queue,  512 KB)
  DMA   accs [i,q,it,j] <- acc   (ACT queue, 512 KB)
  PE    xt = transpose(xs)       (fp16 identity, 16x 128x128)
  ACT   ext = Exp(xt)            (PSUM -> SBUF fp16)
  PE    asum[q,j] = ones.T @ accs (replicated over partitions, f32 PSUM)
  DVE   rrep = 1/asum            (-> fp16 SBUF)
  DVE   accn = accs * rrep       (fp16, softmax-normalized weights, 2x mode)
  PE    ys[b,j] += ext.T @ accn  (fp16 matmul, f32 PSUM accum)
  ACT   outs = Ln(ys)            (-> fp16 SBUF)
  DMA   outs -> out              (queued after all loads)
"""

import numpy as np
from contextlib import ExitStack

import bass_rust as _bass_rust

import concourse.bass as bass
import concourse.mybir as mybir
import concourse.tile as tile
from concourse import bacc, masks
from concourse.bass_utils import run_bass_kernel_spmd
from concourse.hw_specs import get_activation_tables

F16 = mybir.dt.float16
F32 = mybir.dt.float32
AF = mybir.ActivationFunctionType

NUM_SCOPES, NUM_DECOMPS, BATCH, NUM_IN, NUM_SUMS = 32, 8, 256, 256, 256
N_CORES = 8
SCOPES_PER_CORE = NUM_SCOPES // N_CORES          # 4
PAIRS_PER_CORE = SCOPES_PER_CORE * NUM_DECOMPS   # 32
BLK = 4                                          # pairs per DMA/compute block


def emit_densesum(tc, x_ap, a_ap, o_ap, pairs):
    """x_ap: [pairs, 256(b), 256(i)] f16 DRAM
    a_ap: [pairs, 256(i), 256(j)] f16 DRAM
    o_ap: [pairs, 256(b), 256(j)] f16 DRAM
    """
    nc = tc.nc
    assert pairs % BLK == 0
    nblk = pairs // BLK

    with ExitStack() as ctx:
        ep = ctx.enter_context

        const_pool = ep(tc.tile_pool(name="const", bufs=1))
        xs_pool = ep(tc.tile_pool(name="xs", bufs=nblk + 2))
        acc_pool = ep(tc.tile_pool(name="accs", bufs=nblk + 2))
        ext_pool = ep(tc.tile_pool(name="ext", bufs=4))
        rrep_pool = ep(tc.tile_pool(name="rrep", bufs=4))
        accn_pool = ep(tc.tile_pool(name="accn", bufs=4))
        outs_pool = ep(tc.tile_pool(name="outs", bufs=nblk + 3))
        xt_pool = ep(tc.tile_pool(name="xt", bufs=1, space="PSUM"))
        ys_pool = ep(tc.tile_pool(name="ys", bufs=2, space="PSUM"))
        as_pool = ep(tc.tile_pool(name="asum", bufs=1, space="PSUM"))

        ident = const_pool.tile([128, 128], F16)
        masks.make_identity(nc, ident[:])
        # ones.T @ acc replicates the column sums sum_i acc[i,j] across all
        # 128 output partitions, so no partition-broadcast is needed later.
        ones = const_pool.tile([128, 128], F16)
        nc.gpsimd.memset(ones[:], 1.0)

        # Early dummy matmuls: start the PE p-state ramp clock at t~0 so the
        # real matmuls (first visited at ~3.6us, after the first DMA lands)
        # are already past the 3us warm threshold.  Shares the xt pool's
        # buffer; its WAR dep on the first block's transposes resolves ~1us
        # in, long before the first DMA completes.
        warm = xt_pool.tile([128, 128], F32, name="warm", tag="xt")
        for _ in range(2):
            nc.tensor.matmul(warm[:], ones[:], ones[:])

        # pair index -> (ext tile, accn tile, local q within those tiles)
        front_tiles = {}
        store_insts = []

        # All loads are emitted up front, acc before x per group, on the SP
        # queue.  Transfers must be >=728ns (2+ pairs): the HWDGE pipeline
        # (~650ns/instr) can't feed smaller transfers back-to-back and the
        # DMA device — the global bottleneck — would idle between them.
        # Compute below reads *slices* of these group tiles, so compute
        # chunking is decoupled from DMA granularity.
        load_groups = [(0, 2), (2, 2)] + [(b * BLK, BLK) for b in range(1, nblk)]
        xs_of, accs_of = {}, {}   # pair -> (tile, local offset)
        for gi, (g0, gn) in enumerate(load_groups):
            accs = acc_pool.tile([128, gn, 2, 256], F16)  # [i_l, q, it, j]
            xs = xs_pool.tile([128, gn, 2, 256], F16)     # [b_l, q, bt, i]
            a_dma = lambda: nc.sync.dma_start(
                accs[:], a_ap[g0:g0 + gn].rearrange("q (it i) j -> i q it j", i=128)
            )
            x_dma = lambda: nc.sync.dma_start(
                xs[:], x_ap[g0:g0 + gn].rearrange("q (bt b) i -> b q bt i", b=128)
            )
            a_dma(); x_dma()
            for q in range(gn):
                xs_of[g0 + q] = (xs, q)
                accs_of[g0 + q] = (accs, q)

        def emit_front(p0, n):
            """Transpose + exp + asum/recip/accn for pairs p0..p0+n."""
            xs, xq = xs_of[p0]
            accs, aq = accs_of[p0]
            xs = xs[:, xq:xq + n]
            accs = accs[:, aq:aq + n]

            # asum[q, j] = sum_i accs[i, q, :, j], replicated over partitions
            asum = as_pool.tile([128, n, 256], F32)
            for q in range(n):
                for it in range(2):
                    nc.tensor.matmul(
                        asum[:, q, :], ones[:], accs[:, q, it, :],
                        start=(it == 0), stop=(it == 1),
                    )
            # transpose x -> xt [i_l, q, it, bt, b_l] (fp16 PSUM)
            xt = xt_pool.tile([128, n, 2, 2, 128], F16)
            for q in range(n):
                for it in range(2):
                    for bt in range(2):
                        nc.tensor.matmul(
                            xt[:, q, it, bt, :],
                            xs[:, q, bt, it * 128:(it + 1) * 128],
                            ident[:],
                            is_transpose=True,
                        )
            ext = ext_pool.tile([128, n, 2, 2, 128], F16)
            nc.scalar.activation(ext[:], xt[:], AF.Exp)

            rrep = rrep_pool.tile([128, n, 256], F16)
            with nc.allow_low_precision(reason="1/asum fits fp16; rel err 2^-11"):
                nc.vector.reciprocal(rrep[:], asum[:])
            accn = accn_pool.tile([128, n, 2, 256], F16)
            nc.vector.tensor_mul(
                accn[:], accs[:],
                rrep[:].unsqueeze(2).broadcast_to([128, n, 2, 256]),
            )
            for q in range(n):
                front_tiles[p0 + q] = (ext, accn, q)

        def emit_back(p0, n):
            """Main matmuls + Ln + store for pairs p0..p0+n (deferred, see
            below).  2-pair chunks with ys double-buffered (bufs=2): a
            chunk's mms wait on the Ln two chunks back, which is long done,
            so the ACT engine never stalls on the ys ring."""
            outs = outs_pool.tile([128, n, 2, 256], F16)
            ys = ys_pool.tile([128, n, 2, 256], F32)
            for q in range(n):
                ext, accn, lq = front_tiles[p0 + q]
                for bt in range(2):
                    for it in range(2):
                        nc.tensor.matmul(
                            ys[:, q, bt, :],
                            ext[:, lq, it, bt, :],
                            accn[:, lq, it, :],
                            start=(it == 0), stop=(it == 1),
                        )
            nc.scalar.activation(outs[:], ys[:], AF.Ln)
            store_insts.append((p0, n, outs, 0))

        # Front chunks: the first block is split 1+1+2 so the first Ln (and
        # with it the steady ys-cycle cadence) starts ~3us sooner; the rest
        # run at full block granularity.
        fronts = [(0, 2), (2, 2)] + [
            (b * BLK, BLK) for b in range(1, nblk)
        ]
        # Back chunks lag one front chunk (engine SEQs are in-order and hold
        # while waits are pending, so a back chunk's mms/Ln must sit behind
        # the next chunk's independent front work in program order).
        backs = [(p, 2) for p in range(0, pairs, 2)]
        bi = 0
        covered = 0
        for p0, n in fronts:
            prev_covered, covered = covered, covered + n
            emit_front(p0, n)
            while bi < len(backs) and backs[bi][0] + backs[bi][1] <= prev_covered:
                b0, bn = backs[bi]
                emit_back(b0, bn)
                bi += 1
        while bi < len(backs):
            b0, bn = backs[bi]
            emit_back(b0, bn)
            bi += 1

        # Stores are emitted after all loads in program order on the same
        # (SP) queue so a not-yet-ready store never head-of-line blocks a
        # load, and never blocks ACT work either.
        for p0, m, outs, q0 in store_insts:
            nc.sync.dma_start(
                o_ap[p0:p0 + m].rearrange("q (bt b) j -> b q bt j", b=128),
                outs[:, q0:q0 + m, :, :],
            )


class _Bacc(bacc.Bacc):
    """Bacc whose activation-table pass only considers the one set that
    holds both Exp and Ln, avoiding per-switch table loads."""

    def insert_act_table_loads(self):
        has_activation = any(
            isinstance(i, mybir.InstActivation)
            for b in self.main_func.blocks
            for i in b.instructions
        )
        if not has_activation:
            return
        tables = []
        for name, funcs in get_activation_tables(self.m.arch).items():
            if name != "natural_log_exp_and_others":
                funcs = set()
            tables.append((name, funcs))
        _bass_rust.insert_act_table_loads(self, tables)


def build_nc(pairs=PAIRS_PER_CORE):
    nc = _Bacc("TRN2", target_bir_lowering=False, debug=False)
    x_d = nc.dram_tensor("x", [pairs, BATCH, NUM_IN], F16, kind="ExternalInput")
    a_d = nc.dram_tensor("acc", [pairs, NUM_IN, NUM_SUMS], F16, kind="ExternalInput")
    o_d = nc.dram_tensor("out", [pairs, BATCH, NUM_SUMS], F16, kind="ExternalOutput")
    with tile.TileContext(nc) as tc:
        emit_densesum(tc, x_d.ap(), a_d.ap(), o_d.ap(), pairs)
    nc.compile()
    return nc


_NC_CACHE = {}


def _get_nc():
    key = "main"
    if key not in _NC_CACHE:
        _NC_CACHE[key] = build_nc()
    return _NC_CACHE[key]


def kernel(x: np.ndarray, accumulators: np.ndarray) -> np.ndarray:
    assert x.shape == (NUM_SCOPES, NUM_DECOMPS, BATCH, NUM_IN)
    assert accumulators.shape == (NUM_SCOPES, NUM_DECOMPS, NUM_IN, NUM_SUMS)
    nc = _get_nc()
    x16 = np.ascontiguousarray(x, dtype=np.float16)
    a16 = np.ascontiguousarray(accumulators, dtype=np.float16)
    in_maps = []
    for c in range(N_CORES):
        s0 = c * SCOPES_PER_CORE
        s1 = s0 + SCOPES_PER_CORE
        in_maps.append({
            "x": x16[s0:s1].reshape(PAIRS_PER_CORE, BATCH, NUM_IN),
            "acc": a16[s0:s1].reshape(PAIRS_PER_CORE, NUM_IN, NUM_SUMS),
        })
    res = run_bass_kernel_spmd(nc, in_maps, core_ids=list(range(N_CORES)))
    outs = [
        res.results[c]["out"].reshape(
            SCOPES_PER_CORE, NUM_DECOMPS, BATCH, NUM_SUMS
        )
        for c in range(N_CORES)
    ]
    return np.concatenate(outs, axis=0).astype(np.float32)
